# revision 20
# baseline (speedup 1.0000x reference)
"""Trainium2 Bass kernel for nn_DCWTv2InferenceCache (segment-tree cached attention).

Sharding: tensor-parallel over the 16-head axis -> 8 cores x 2 heads.
Each core streams its (50000, 2*64) f32 slice of the value cache from HBM,
reduces segment-tree nodes to (64, 128) block-sums on the PE (selection-matrix
matmul accumulating in PSUM), then runs the per-node depth-projected attention
epilogue fully on-device. Output is head-sharded (2, 64) per core, gathered on
host. No cross-device communication.

Schedule notes (v2):
 - The big token stream is the only thing on the sync HWDGE ring and its
   DMAs are issued first, so the 16 DMA engines saturate right after the
   runtime preamble instead of ~10us in.
 - All small prefetches (cblob, local window, tail nodes) ride the scalar
   HWDGE ring and land under the first stream DMA.
 - The two selection matrices are built on-device (3 vector ops) instead of
   a 2.1 MB HBM blob: the sel64 diagonal lives at stride 65 in a (128, 4160)
   tile, so one strided memset writes all 64 ones.
 - Stream DMAs are quarter-r splits of a 2-chunk block (2.1 MB each, 8 KB
   descriptors): each is chased by 16 matmuls with 256-wide outputs (the
   f32r fast path), so after the last DMA lands only ~16 matmuls + one
   epilogue remain.
"""

import math
import sys

if "/opt/trn_rl_repo" not in sys.path:
    sys.path.insert(0, "/opt/trn_rl_repo")

import numpy as np

import concourse.bass as bass
import concourse.mybir as mybir
import concourse.tile as tile
from concourse import bacc
from concourse.bass_utils import run_bass_kernel_spmd

# --- problem constants (from the reference nn.Module) ---
MAX_LEN = 65536
NUM_HEADS = 16
HEAD_DIM = 64
K_MAX = 64
LOCAL_WINDOW = 512
LOG_N = 17
LEAF_START = 2**LOG_N

N_CORES = 8
HPC = NUM_HEADS // N_CORES        # heads per core = 2
F = HPC * HEAD_DIM                # feature width per core = 128
NTOK = 50000                      # v_tokens buffer length

CHUNK = 128                       # tokens per matmul tile (partition dim)
BLK = CHUNK * K_MAX               # 8192 tokens per linear c-chunk
RSPLIT = 4                        # r-ranges per 2-chunk stream unit
STREAM_BUFS = 8                   # in-flight stream unit tiles

STAGE_A_MODE = "r64"              # kept for test.py compat


def _cblob_layout(NT):
    """Column offsets inside the packed (128, W) f32 constants blob."""
    nt = max(NT, 1)
    off = {}
    off["ident"] = 0
    off["qbd"] = 128
    off["qT"] = 130
    off["temps"] = 132
    off["wTI"] = 132 + nt
    return off, 132 + nt + nt * 64

f32 = mybir.dt.float32
f32r = mybir.dt.float32r
AF = mybir.ActivationFunctionType
AX = mybir.AxisListType

_last_results = None  # stash for test harness introspection


def cover_set(pos):
    """O(log n) segment-tree nodes covering prefix [0..pos-1]: (start, L, depth),
    ascending start / descending L (binary decomposition of pos)."""
    if pos <= 0:
        return []
    l, r = LEAF_START, LEAF_START + min(pos, MAX_LEN)
    out = []
    while l < r:
        if l & 1:
            d = LOG_N - int(math.floor(math.log2(l)))
            out.append(((l << d) - LEAF_START, 1 << d, d))
            l += 1
        if r & 1:
            r -= 1
            d = LOG_N - int(math.floor(math.log2(r)))
            out.append(((r << d) - LEAF_START, 1 << d, d))
        l >>= 1
        r >>= 1
    return sorted(out)


def _build_program(pos, mode="r64"):
    """Build the single-core Bass/Tile program (same program for all 8 cores)."""
    nodes = cover_set(pos)
    big = [(s, L, d) for (s, L, d) in nodes if L > K_MAX]      # L >= 128, 128-aligned
    small = [(s, L, d) for (s, L, d) in nodes if L <= K_MAX]   # raw tail nodes
    tree = big + small                                          # epilogue order
    NT = len(tree)
    n_loc = min(pos, LOCAL_WINDOW)
    assert n_loc % CHUNK == 0, "local window must be chunk-aligned for this build"
    NLC = n_loc // CHUNK

    inv_sqrt_d = 1.0 / math.sqrt(HEAD_DIM)

    strm = [i for i in range(len(big)) if big[i][1] >= BLK]    # streamed nodes
    old = [i for i in range(len(big)) if big[i][1] < BLK]      # PE-folded, prefetch

    # stream unit list: (node_i, c0, ncc, r0, nr, first_of_node, last_of_node)
    units = []
    for ni in strm:
        start_b, L_b, _d = big[ni]
        CC = L_b // BLK
        pairs = [(c0, min(2, CC - c0)) for c0 in range(0, CC, 2)]
        for pi, (c0, ncc) in enumerate(pairs):
            if ncc == 2:
                nr = K_MAX // RSPLIT
                for s in range(RSPLIT):
                    units.append(dict(ni=ni, c0=c0, ncc=2, r0=s * nr, nr=nr,
                                      first=(pi == 0 and s == 0),
                                      last=(pi == len(pairs) - 1 and s == RSPLIT - 1)))
            else:  # odd trailing chunk: no r-split (keeps 128-wide out, slow path ok)
                units.append(dict(ni=ni, c0=c0, ncc=1, r0=0, nr=K_MAX,
                                  first=(pi == 0), last=(pi == len(pairs) - 1)))

    nc = bacc.Bacc("TRN2", target_bir_lowering=False, debug=False)

    v = nc.dram_tensor("v", [NTOK, F], f32, kind="ExternalInput")
    CBOFF, CB_W = _cblob_layout(NT)
    CB_IDENT, CB_QBD, CB_QT = CBOFF["ident"], CBOFF["qbd"], CBOFF["qT"]
    CB_TEMPS, CB_WTI = CBOFF["temps"], CBOFF["wTI"]
    cblob_d = nc.dram_tensor("cblob", [CHUNK, CB_W], f32, kind="ExternalInput")
    o = nc.dram_tensor("o", [HPC, HEAD_DIM], f32, kind="ExternalOutput")

    with tile.TileContext(nc) as tc:
        with (
            tc.tile_pool(name="consts", bufs=1) as cpool,
            tc.tile_pool(name="vstream", bufs=STREAM_BUFS) as vpool,
            tc.tile_pool(name="fsb", bufs=2) as fpool,
            tc.tile_pool(name="ep_sb", bufs=2) as spool,
            tc.tile_pool(name="xsb", bufs=3) as xpool,
            tc.tile_pool(name="acc_ps", bufs=1, space=bass.MemorySpace.PSUM) as apool,
            tc.tile_pool(name="ep_ps", bufs=1, space=bass.MemorySpace.PSUM) as eppool,
            tc.tile_pool(name="out_ps", bufs=1, space=bass.MemorySpace.PSUM) as opool,
        ):
            # ================= stream head: sync ring starts NOW =================
            def unit_dma(u):
                ut = vpool.tile([CHUNK, 2, u["nr"] * F], f32r, tag="vbig")
                start_b = big[u["ni"]][0]
                src = v[start_b + u["c0"] * BLK :
                        start_b + (u["c0"] + u["ncc"]) * BLK, :]
                src = src.bitcast(f32r)
                src = src.rearrange("(c q r) f -> q c (r f)", q=CHUNK, r=K_MAX)
                nc.sync.dma_start(
                    ut[:, 0 : u["ncc"], :],
                    src[:, :, u["r0"] * F : (u["r0"] + u["nr"]) * F],
                )
                return ut

            unit_tiles = {}
            n_head = min(STREAM_BUFS, len(units))
            for k in range(n_head):
                unit_tiles[k] = unit_dma(units[k])

            # ================= constants: scalar HWDGE ring =================
            cb = cpool.tile([CHUNK, CB_W], f32)
            nc.scalar.dma_start(cb[:], cblob_d[:])
            ident_sb = cb[:, CB_IDENT : CB_IDENT + CHUNK]
            qbd_sb = cb[:, CB_QBD : CB_QBD + HPC]
            qT_sb = cb[0:HEAD_DIM, CB_QT : CB_QT + HPC]
            temps2_sb = cb[0:HPC, CB_TEMPS : CB_TEMPS + max(NT, 1)]

            def wTI_slice(n):
                return cb[0:HEAD_DIM, CB_WTI + n * HEAD_DIM : CB_WTI + (n + 1) * HEAD_DIM]

            # raw tail data (small nodes + local window)
            small_tiles = []
            for si, (start_s, L_s, _d) in enumerate(small):
                fsm = cpool.tile([K_MAX, F], f32, name=f"fsm{si}", tag=f"fsm{si}")
                nc.scalar.dma_start(fsm[0:L_s, :], v[start_s : start_s + L_s, :])
                small_tiles.append(fsm)
            lstart = pos - n_loc
            fl_sb = cpool.tile([CHUNK, NLC, F], f32)
            nc.scalar.dma_start(
                fl_sb[:],
                v[lstart : lstart + n_loc, :].rearrange("(c p) f -> p c f", p=CHUNK),
            )
            # small big-nodes (old path): one 3-D DMA per node, plain f32
            # (their 2-3 fold matmuls are tiny and off the critical path)
            oldpath_tiles = {}
            for i in old:
                start_b, L_b, _d = big[i]
                nch_b = L_b // CHUNK
                vo = cpool.tile([CHUNK, nch_b, F], f32,
                                name=f"vo{start_b}", tag=f"vo{start_b}")
                nc.scalar.dma_start(
                    vo[:],
                    v[start_b : start_b + L_b, :].rearrange("(c p) f -> p c f", p=CHUNK),
                )
                oldpath_tiles[start_b] = vo

            # ================= on-device selection matrices =================
            # sel64 blob: lhsT(r) = selB[:, r*64 : r*64+64] has ones in column r,
            # i.e. flat nonzeros at stride-65 positions 0, 65, 130, ...
            # Built with f32 memsets, then one SBUF->SBUF DMA into the f32r
            # tile (only DMAs may produce f32r-consumed memory).
            selF = cpool.tile([CHUNK, K_MAX * 65], f32)
            nc.vector.memset(selF[:], 0.0)
            diag = selF[:].rearrange("p (r s) -> p r s", s=65)
            nc.vector.memset(diag[:, :, 0:1], 1.0)
            selB = cpool.tile([CHUNK, K_MAX * 65], f32r)
            nc.scalar.dma_start(selB[:], selF[:].bitcast(f32r))

            def sel64_lhsT(r):
                return selB[:, r * K_MAX : (r + 1) * K_MAX]

            # old-path fold matrix sel[q, j] = (q % 64 == j), from the identity
            sel_f32 = cpool.tile([CHUNK, K_MAX], f32)
            nc.vector.tensor_add(
                sel_f32[:], ident_sb[:, 0:K_MAX], ident_sb[:, K_MAX : 2 * K_MAX]
            )
            sel_sb = sel_f32[:]

            # ---- per-node softmax scales: 1/((softplus(t)+1e-6)*sqrt(D)) ----
            et_sb = cpool.tile([HPC, max(NT, 1)], f32)
            nc.scalar.activation(et_sb[:], temps2_sb, AF.Exp)
            sp_sb = cpool.tile([HPC, max(NT, 1)], f32)
            nc.scalar.activation(sp_sb[:], et_sb[:], AF.Ln, bias=1.0)  # softplus
            u_sb = cpool.tile([HPC, max(NT, 1)], f32)
            # u = (sp + 1e-6) * sqrt(D) = sp*sqrt(D) + 1e-6*sqrt(D)
            nc.scalar.mul(u_sb[:], sp_sb, math.sqrt(HEAD_DIM))
            nc.vector.tensor_scalar_add(u_sb[:], u_sb[:], 1e-6 * math.sqrt(HEAD_DIM))
            rs_sb = cpool.tile([HPC, max(NT, 1)], f32)
            nc.vector.reciprocal(rs_sb[:], u_sb[:])
            ns_sb = cpool.tile([HPC, max(NT, 1)], f32)
            nc.scalar.mul(ns_sb[:], rs_sb[:], -1.0)

            # ---- all tree-node q_depth projections upfront (block-diag) ----
            # all 2*NT little matmuls land in one PSUM tile, then one copy:
            # no PE<->ACT ping-pong at the head of the PE queue.
            qd_all = cpool.tile([2 * HEAD_DIM, max(NT, 1), HPC], f32)
            nc.vector.memset(qd_all[:], 0.0)
            qd_ps = eppool.tile([2 * HEAD_DIM, max(NT, 1), HPC], f32, tag="qd_ps")
            for n in range(NT):
                nc.tensor.matmul(
                    qd_ps[0:HEAD_DIM, n, 0:1],
                    wTI_slice(n), qT_sb[:, 0:1], start=True, stop=True,
                )
                nc.tensor.matmul(
                    qd_ps[HEAD_DIM : 2 * HEAD_DIM, n, 1:2],
                    wTI_slice(n), qT_sb[:, 1:2], start=True, stop=True,
                )
            for n in range(NT):
                nc.scalar.copy(qd_all[0:HEAD_DIM, n, 0:1], qd_ps[0:HEAD_DIM, n, 0:1])
                nc.scalar.copy(
                    qd_all[HEAD_DIM : 2 * HEAD_DIM, n, 1:2],
                    qd_ps[HEAD_DIM : 2 * HEAD_DIM, n, 1:2],
                )

            # ---- cross-node output accumulator (2, 128) PSUM ----
            out_ps = opool.tile([HPC, F], f32)
            n_out_mm = len(tree) + NLC
            out_mm = [0]  # running count, for start/stop flags

            def out_matmul(wT_sb_ap, f_sb_ap):
                nc.tensor.matmul(
                    out_ps[:], wT_sb_ap, f_sb_ap,
                    start=(out_mm[0] == 0), stop=(out_mm[0] == n_out_mm - 1),
                )
                out_mm[0] += 1

            def softmax_weights(s_ps_ap, K, node_i, is_tree):
                """softmax over K free-dim entries of (2, K) logits (pre-scale);
                returns (2, K) SBUF weights; tree weights folded by 1/NT."""
                smax = xpool.tile([HPC, 1], f32, tag="smax")
                nc.vector.reduce_max(smax[:], s_ps_ap, axis=AX.X)
                biast = xpool.tile([HPC, 1], f32, tag="biast")
                ebd = xpool.tile([HPC, K], f32, tag="esb")
                zt = xpool.tile([HPC, 1], f32, tag="zt")
                if is_tree:
                    nc.vector.tensor_scalar_mul(
                        biast[:], smax[:], ns_sb[:, node_i : node_i + 1]
                    )
                    nc.scalar.activation(
                        ebd[:], s_ps_ap, AF.Exp,
                        bias=biast[:], scale=rs_sb[:, node_i : node_i + 1],
                        accum_out=zt[:],
                    )
                else:
                    nc.scalar.mul(biast[:], smax[:], -inv_sqrt_d)
                    nc.scalar.activation(
                        ebd[:], s_ps_ap, AF.Exp, bias=biast[:], scale=inv_sqrt_d,
                        accum_out=zt[:],
                    )
                if is_tree:
                    zs = xpool.tile([HPC, 1], f32, tag="zs")
                    nc.scalar.mul(zs[:], zt[:], float(NT))
                    zt = zs
                rz = xpool.tile([HPC, 1], f32, tag="rz")
                nc.vector.reciprocal(rz[:], zt[:])
                w_sb = xpool.tile([HPC, K], f32, tag="wsb")
                nc.vector.tensor_scalar_mul(w_sb[:], ebd[:], rz[:])
                return w_sb

            def tree_ep_stages(node_i, f_ap, K):
                """Attention epilogue split into 4 stages, one PE touch each.
                Stages run in CONSECUTIVE stream gaps, so each stage's
                scalar/vector inputs were produced a full DMA-unit (~5us)
                earlier and the PE never blocks on a cross-engine wait."""
                st = {}

                def s0():  # fT (F, K) for the logits matmul
                    fT_ps = eppool.tile([F, K_MAX], f32, tag="fT_ps")
                    nc.tensor.transpose(fT_ps[:, 0:K], f_ap, ident_sb[0:K, 0:K])
                    fT_sb = spool.tile([F, K_MAX], f32, tag="fT_sb")
                    nc.scalar.copy(fT_sb[:, 0:K], fT_ps[:, 0:K])
                    st["fT"] = fT_sb

                def s1():  # logits (2, K) + softmax on vec/scalar
                    s_ps = eppool.tile([HPC, K_MAX], f32, tag="s_ps", bufs=2)
                    nc.tensor.matmul(
                        s_ps[:, 0:K], qd_all[:, node_i, :], st["fT"][:, 0:K],
                        start=True, stop=True,
                    )
                    st["w"] = softmax_weights(s_ps[:, 0:K], K, node_i, True)

                def s2():  # wT for the output matmul
                    wT_ps = eppool.tile([K_MAX, HPC], f32, tag="wT_ps")
                    nc.tensor.transpose(
                        wT_ps[0:K, :], st["w"][:], ident_sb[0:HPC, 0:HPC]
                    )
                    wT_sb = spool.tile([K_MAX, HPC], f32, tag="wT_sb")
                    nc.scalar.copy(wT_sb[0:K, :], wT_ps[0:K, :])
                    st["wT"] = wT_sb

                def s3():  # out += wT.T @ f (folded by 1/NT via Z scaling)
                    out_matmul(st["wT"][0:K, :], f_ap)

                return [s0, s1, s2, s3]

            def node_stage_f(ps2, L, CC2):
                """psum (64, 2, F) -> mean-scaled f_sb tile."""
                f_sb = fpool.tile([K_MAX, F], f32, tag="f")
                mean_scale = float(K_MAX) / L
                if CC2 > 1:
                    ha = fpool.tile([K_MAX, F], f32, tag="ha")
                    nc.scalar.mul(ha[:], ps2[:, 0, :], mean_scale)
                    hb = fpool.tile([K_MAX, F], f32, tag="hb")
                    nc.scalar.mul(hb[:], ps2[:, 1, :], mean_scale)
                    nc.vector.tensor_add(f_sb[:], ha[:], hb[:])
                else:
                    nc.scalar.mul(f_sb[:], ps2[:, 0, :], mean_scale)
                return f_sb

            def emit_old_stage_a(node_i, start, L):
                """Fold matmuls + psum->f_sb for an old node; cheap on PE, so
                it runs pre-stream.  The ping-pong epilogue is deferred."""
                nch = L // CHUNK
                vt = oldpath_tiles[start]
                ps2 = apool.tile([K_MAX, 2, F], f32, tag="acc")
                done = 0
                c = 0
                while c < nch:
                    w = 2 if c + 2 <= nch else 1
                    nc.tensor.matmul(
                        ps2[:, 0:w, :], sel_sb, vt[:, c : c + w, :],
                        start=(done == 0), stop=(done + w == nch),
                    )
                    done += w
                    c += w
                return node_stage_f(ps2, L, nch)

            def local_ep_stages():
                st = {}

                def s0():
                    fTl_ps = eppool.tile([F, NLC * CHUNK], f32, tag="fT_ps")
                    for c in range(NLC):
                        nc.tensor.transpose(
                            fTl_ps[:, c * CHUNK : (c + 1) * CHUNK], fl_sb[:, c, :],
                            ident_sb[:],
                        )
                    fTl_sb = spool.tile([F, NLC * CHUNK], f32, tag="fTl_sb")
                    nc.scalar.copy(fTl_sb[:], fTl_ps[:])
                    st["fT"] = fTl_sb

                def s1():
                    sl_ps = eppool.tile([HPC, NLC * CHUNK], f32, tag="s_ps", bufs=2)
                    nc.tensor.matmul(
                        sl_ps[:], qbd_sb, st["fT"][:], start=True, stop=True
                    )
                    st["w"] = softmax_weights(sl_ps[:], n_loc, -1, False)

                def s2():
                    wTl_ps = eppool.tile([CHUNK, NLC, HPC], f32, tag="wT_ps")
                    for c in range(NLC):
                        nc.tensor.transpose(
                            wTl_ps[:, c, :], st["w"][:, c * CHUNK : (c + 1) * CHUNK],
                            ident_sb[0:HPC, 0:HPC],
                        )
                    wTl_sb = spool.tile([CHUNK, NLC, HPC], f32, tag="wTl_sb")
                    nc.scalar.copy(wTl_sb[:], wTl_ps[:])
                    st["wT"] = wTl_sb

                def s3():
                    for c in range(NLC):
                        out_matmul(st["wT"][:, c, :], fl_sb[:, c, :])

                return [s0, s1, s2, s3]

            # ---- old-node fold matmuls pre-stream (the shared acc PSUM slot
            # must be past its last reader before stream node 0 claims it) ----
            old_f = {i: emit_old_stage_a(i, big[i][0], big[i][1]) for i in old}

            # ---- epilogue pipeline: one new epilogue activates per stream
            # gap, and each active epilogue advances one stage per gap ----
            to_activate = [local_ep_stages()]
            to_activate += [
                tree_ep_stages(len(big) + si, small_tiles[si][0 : small[si][1], :],
                               small[si][1])
                for si in range(len(small))
            ]
            to_activate += [tree_ep_stages(i, old_f[i][:], K_MAX) for i in old]
            to_activate.reverse()
            pipe = []

            def pump_stages():
                if to_activate:
                    pipe.append(to_activate.pop())
                for sl in pipe:
                    sl.pop(0)()
                pipe[:] = [sl for sl in pipe if sl]

            # PE keep-warm: garbage matmuls into a dead PSUM bank so the
            # Tensor engine's p-state never drops back during DMA waits
            # (ramped 2.4 GHz vs 1.2 GHz is the stream-matmul cadence).
            warm_ps = eppool.tile([K_MAX, 2, F], f32, tag="warm")
            WARM_N = 12

            def pump_warm(n):
                for _ in range(n):
                    nc.tensor.matmul(
                        warm_ps[:], sel64_lhsT(0), selB[:, 0:2 * F],
                        start=True, stop=True, skip_group_check=True,
                    )

            # ================= streamed nodes =================
            node_mm_total = {}
            for u in units:
                node_mm_total[u["ni"]] = node_mm_total.get(u["ni"], 0) + u["nr"]
            node_mm_done = {ni: 0 for ni in node_mm_total}
            node_ps = {}

            for k, u in enumerate(units):
                ni = u["ni"]
                ut = unit_tiles[k]
                if u["first"]:
                    acc_ps = apool.tile([K_MAX, 2, F], f32, tag="acc")
                    node_ps[ni] = acc_ps
                ps2 = node_ps[ni]
                done = node_mm_done[ni]
                tot = node_mm_total[ni]
                for j in range(u["nr"]):
                    r = u["r0"] + j
                    nc.tensor.matmul(
                        ps2[:, 0 : u["ncc"], :], sel64_lhsT(r),
                        ut[:, 0 : u["ncc"], j * F : (j + 1) * F],
                        start=(done == 0), stop=(done == tot - 1),
                    )
                    done += 1
                node_mm_done[ni] = done
                # refill the stream pipeline (slot k % STREAM_BUFS now has
                # known readers, so the WAR semaphore is correct)
                if k + STREAM_BUFS < len(units):
                    unit_tiles[k + STREAM_BUFS] = unit_dma(units[k + STREAM_BUFS])
                if u["last"]:
                    start_b, L_b, _d = big[ni]
                    f_sb = node_stage_f(ps2, L_b, 2 if u["ncc"] == 2 else L_b // BLK)
                    to_activate.insert(0, tree_ep_stages(ni, f_sb[:], K_MAX))
                pump_stages()
                if k < len(units) - 2:
                    pump_warm(WARM_N)
            # drain remaining epilogue stages (the last node's whole chain)
            while to_activate or pipe:
                pump_stages()

            # ================= final output =================
            acc_sb = spool.tile([HPC, F], f32, tag="acc_sb")
            nc.scalar.copy(acc_sb[:], out_ps[:])
            # head h's output lives at acc_sb[h, h*64:(h+1)*64]; DMA handles the
            # partition-base-1 read that compute engines can't.
            nc.sync.dma_start(o[0:1, :], acc_sb[0:1, 0:HEAD_DIM])
            nc.sync.dma_start(o[1:2, :], acc_sb[1:2, HEAD_DIM : 2 * HEAD_DIM])

    nc.compile()
    return nc


def _make_in_maps(v_tokens, q_new, depth_proj_w, depth_temp, pos):
    nodes = cover_set(pos)
    big = [(st, L, d) for (st, L, d) in nodes if L > K_MAX]
    small = [(st, L, d) for (st, L, d) in nodes if L <= K_MAX]
    tree = big + small
    NT = len(tree)
    OFF, CB_W = _cblob_layout(NT)

    wTI = np.stack(
        [np.eye(HEAD_DIM, dtype=np.float32) + depth_proj_w[d].T for (_, _, d) in tree]
    ) if NT else np.zeros((1, HEAD_DIM, HEAD_DIM), np.float32)
    tsel = np.array([depth_temp[d] for (_, _, d) in tree], np.float32) \
        if NT else np.zeros((1,), np.float32)

    in_maps = []
    for c in range(N_CORES):
        q_c = q_new[0, HPC * c : HPC * (c + 1), :]          # (2, 64)
        cb = np.zeros((CHUNK, CB_W), np.float32)
        cb[:, OFF["ident"] : OFF["ident"] + CHUNK] = np.eye(CHUNK)
        for h in range(HPC):
            cb[h * HEAD_DIM : (h + 1) * HEAD_DIM, OFF["qbd"] + h] = q_c[h]
        cb[0:HEAD_DIM, OFF["qT"] : OFF["qT"] + HPC] = q_c.T
        cb[0:HPC, OFF["temps"] : OFF["temps"] + max(NT, 1)] = tsel[None, :]
        for n in range(max(NT, 1)):
            cb[0:HEAD_DIM, OFF["wTI"] + n * HEAD_DIM : OFF["wTI"] + (n + 1) * HEAD_DIM] = (
                wTI[n] if NT else 0.0
            )
        im = {
            "v": np.ascontiguousarray(
                v_tokens[:, HPC * c : HPC * (c + 1), :]
            ).reshape(NTOK, F),
            "cblob": cb,
        }
        in_maps.append(im)
    return in_maps


def kernel(v_tokens, q_new, depth_proj_w, depth_temp, n_tokens, _profile=False):
    global _last_results
    v_tokens = np.asarray(v_tokens, dtype=np.float32)
    q_new = np.asarray(q_new, dtype=np.float32)
    depth_proj_w = np.asarray(depth_proj_w, dtype=np.float32)
    depth_temp = np.asarray(depth_temp, dtype=np.float32)
    pos = int(n_tokens)

    nc = _build_program(pos)
    in_maps = _make_in_maps(v_tokens, q_new, depth_proj_w, depth_temp, pos)
    res = run_bass_kernel_spmd(
        nc, in_maps, core_ids=list(range(N_CORES)), trace=_profile
    )
    _last_results = res

    out = np.zeros((1, NUM_HEADS, HEAD_DIM), np.float32)
    for c in range(N_CORES):
        out[0, HPC * c : HPC * (c + 1), :] = res.results[c]["o"]
    return out


# revision 25
# speedup vs baseline: 1.0413x; 1.0413x over previous
"""Trainium2 Bass kernel for nn_DCWTv2InferenceCache (segment-tree cached attention).

Sharding: tensor-parallel over the 16-head axis -> 8 cores x 2 heads.
Each core streams its (50000, 2*64) f32 slice of the value cache from HBM,
reduces segment-tree nodes to (64, 128) block-sums on the PE (selection-matrix
matmul accumulating in PSUM), then runs the per-node depth-projected attention
epilogue fully on-device. Output is head-sharded (2, 64) per core, gathered on
host. No cross-device communication.

Schedule notes (v2):
 - The big token stream is the only thing on the sync HWDGE ring and its
   DMAs are issued first, so the 16 DMA engines saturate right after the
   runtime preamble instead of ~10us in.
 - All small prefetches (cblob, local window, tail nodes) ride the scalar
   HWDGE ring and land under the first stream DMA.
 - The two selection matrices are built on-device (3 vector ops) instead of
   a 2.1 MB HBM blob: the sel64 diagonal lives at stride 65 in a (128, 4160)
   tile, so one strided memset writes all 64 ones.
 - Stream DMAs are quarter-r splits of a 2-chunk block (2.1 MB each, 8 KB
   descriptors): each is chased by 16 matmuls with 256-wide outputs (the
   f32r fast path), so after the last DMA lands only ~16 matmuls + one
   epilogue remain.
"""

import math
import sys

if "/opt/trn_rl_repo" not in sys.path:
    sys.path.insert(0, "/opt/trn_rl_repo")

import numpy as np

import concourse.bass as bass
import concourse.mybir as mybir
import concourse.tile as tile
from concourse import bacc
from concourse.bass_utils import run_bass_kernel_spmd

# --- problem constants (from the reference nn.Module) ---
MAX_LEN = 65536
NUM_HEADS = 16
HEAD_DIM = 64
K_MAX = 64
LOCAL_WINDOW = 512
LOG_N = 17
LEAF_START = 2**LOG_N

N_CORES = 8
HPC = NUM_HEADS // N_CORES        # heads per core = 2
F = HPC * HEAD_DIM                # feature width per core = 128
NTOK = 50000                      # v_tokens buffer length

CHUNK = 128                       # tokens per matmul tile (partition dim)
BLK = CHUNK * K_MAX               # 8192 tokens per linear c-chunk
RSPLIT = 4                        # r-ranges per 2-chunk stream unit
STREAM_BUFS = 8                   # in-flight stream unit tiles

STAGE_A_MODE = "r64"              # kept for test.py compat


def _cblob_layout(NT):
    """Column offsets inside the packed (128, W) f32 constants blob."""
    nt = max(NT, 1)
    off = {}
    off["ident"] = 0
    off["qbd"] = 128
    off["qT"] = 130
    off["temps"] = 132
    off["wTI"] = 132 + nt
    return off, 132 + nt + nt * 64

f32 = mybir.dt.float32
f32r = mybir.dt.float32r
AF = mybir.ActivationFunctionType
AX = mybir.AxisListType

_last_results = None  # stash for test harness introspection


def cover_set(pos):
    """O(log n) segment-tree nodes covering prefix [0..pos-1]: (start, L, depth),
    ascending start / descending L (binary decomposition of pos)."""
    if pos <= 0:
        return []
    l, r = LEAF_START, LEAF_START + min(pos, MAX_LEN)
    out = []
    while l < r:
        if l & 1:
            d = LOG_N - int(math.floor(math.log2(l)))
            out.append(((l << d) - LEAF_START, 1 << d, d))
            l += 1
        if r & 1:
            r -= 1
            d = LOG_N - int(math.floor(math.log2(r)))
            out.append(((r << d) - LEAF_START, 1 << d, d))
        l >>= 1
        r >>= 1
    return sorted(out)


def _build_program(pos, mode="r64"):
    """Build the single-core Bass/Tile program (same program for all 8 cores)."""
    nodes = cover_set(pos)
    big = [(s, L, d) for (s, L, d) in nodes if L > K_MAX]      # L >= 128, 128-aligned
    small = [(s, L, d) for (s, L, d) in nodes if L <= K_MAX]   # raw tail nodes
    tree = big + small                                          # epilogue order
    NT = len(tree)
    n_loc = min(pos, LOCAL_WINDOW)
    assert n_loc % CHUNK == 0, "local window must be chunk-aligned for this build"
    NLC = n_loc // CHUNK

    inv_sqrt_d = 1.0 / math.sqrt(HEAD_DIM)

    strm = [i for i in range(len(big)) if big[i][1] >= BLK]    # streamed nodes
    old = [i for i in range(len(big)) if big[i][1] < BLK]      # PE-folded, prefetch

    # stream unit list: (node_i, c0, ncc, r0, nr, first_of_node, last_of_node)
    units = []
    for ni in strm:
        start_b, L_b, _d = big[ni]
        CC = L_b // BLK
        pairs = [(c0, min(2, CC - c0)) for c0 in range(0, CC, 2)]
        for pi, (c0, ncc) in enumerate(pairs):
            if ncc == 2:
                nr = K_MAX // RSPLIT
                for s in range(RSPLIT):
                    units.append(dict(ni=ni, c0=c0, ncc=2, r0=s * nr, nr=nr,
                                      first=(pi == 0 and s == 0),
                                      last=(pi == len(pairs) - 1 and s == RSPLIT - 1)))
            else:  # odd trailing chunk: no r-split (keeps 128-wide out, slow path ok)
                units.append(dict(ni=ni, c0=c0, ncc=1, r0=0, nr=K_MAX,
                                  first=(pi == 0), last=(pi == len(pairs) - 1)))

    nc = bacc.Bacc("TRN2", target_bir_lowering=False, debug=False)

    v = nc.dram_tensor("v", [NTOK, F], f32, kind="ExternalInput")
    CBOFF, CB_W = _cblob_layout(NT)
    CB_IDENT, CB_QBD, CB_QT = CBOFF["ident"], CBOFF["qbd"], CBOFF["qT"]
    CB_TEMPS, CB_WTI = CBOFF["temps"], CBOFF["wTI"]
    cblob_d = nc.dram_tensor("cblob", [CHUNK, CB_W], f32, kind="ExternalInput")
    o = nc.dram_tensor("o", [HPC, HEAD_DIM], f32, kind="ExternalOutput")

    with tile.TileContext(nc) as tc:
        with (
            tc.tile_pool(name="consts", bufs=1) as cpool,
            tc.tile_pool(name="vstream", bufs=STREAM_BUFS) as vpool,
            tc.tile_pool(name="fsb", bufs=2) as fpool,
            tc.tile_pool(name="ep_sb", bufs=2) as spool,
            tc.tile_pool(name="xsb", bufs=3) as xpool,
            tc.tile_pool(name="acc_ps", bufs=1, space=bass.MemorySpace.PSUM) as apool,
            tc.tile_pool(name="ep_ps", bufs=1, space=bass.MemorySpace.PSUM) as eppool,
            tc.tile_pool(name="out_ps", bufs=1, space=bass.MemorySpace.PSUM) as opool,
        ):
            # ================= stream head: sync ring starts NOW =================
            def unit_dma(u):
                ut = vpool.tile([CHUNK, 2, u["nr"] * F], f32r, tag="vbig")
                start_b = big[u["ni"]][0]
                src = v[start_b + u["c0"] * BLK :
                        start_b + (u["c0"] + u["ncc"]) * BLK, :]
                src = src.bitcast(f32r)
                src = src.rearrange("(c q r) f -> q c (r f)", q=CHUNK, r=K_MAX)
                nc.sync.dma_start(
                    ut[:, 0 : u["ncc"], :],
                    src[:, :, u["r0"] * F : (u["r0"] + u["nr"]) * F],
                )
                return ut

            unit_tiles = {}
            n_head = min(STREAM_BUFS, len(units))
            for k in range(n_head):
                unit_tiles[k] = unit_dma(units[k])

            # ================= constants: scalar HWDGE ring =================
            cb = cpool.tile([CHUNK, CB_W], f32)
            nc.scalar.dma_start(cb[:], cblob_d[:])
            ident_sb = cb[:, CB_IDENT : CB_IDENT + CHUNK]
            qbd_sb = cb[:, CB_QBD : CB_QBD + HPC]
            qT_sb = cb[0:HEAD_DIM, CB_QT : CB_QT + HPC]
            temps2_sb = cb[0:HPC, CB_TEMPS : CB_TEMPS + max(NT, 1)]

            def wTI_slice(n):
                return cb[0:HEAD_DIM, CB_WTI + n * HEAD_DIM : CB_WTI + (n + 1) * HEAD_DIM]

            # raw tail data (small nodes + local window)
            small_tiles = []
            for si, (start_s, L_s, _d) in enumerate(small):
                fsm = cpool.tile([K_MAX, F], f32, name=f"fsm{si}", tag=f"fsm{si}")
                nc.scalar.dma_start(fsm[0:L_s, :], v[start_s : start_s + L_s, :])
                small_tiles.append(fsm)
            lstart = pos - n_loc
            fl_sb = cpool.tile([CHUNK, NLC, F], f32)
            nc.scalar.dma_start(
                fl_sb[:],
                v[lstart : lstart + n_loc, :].rearrange("(c p) f -> p c f", p=CHUNK),
            )
            # small big-nodes (old path): one 3-D DMA per node, plain f32
            # (their 2-3 fold matmuls are tiny and off the critical path)
            oldpath_tiles = {}
            for i in old:
                start_b, L_b, _d = big[i]
                nch_b = L_b // CHUNK
                vo = cpool.tile([CHUNK, nch_b, F], f32,
                                name=f"vo{start_b}", tag=f"vo{start_b}")
                nc.scalar.dma_start(
                    vo[:],
                    v[start_b : start_b + L_b, :].rearrange("(c p) f -> p c f", p=CHUNK),
                )
                oldpath_tiles[start_b] = vo

            # ================= on-device selection matrices =================
            # sel64 blob: lhsT(r) = selB[:, r*64 : r*64+64] has ones in column r,
            # i.e. flat nonzeros at stride-65 positions 0, 65, 130, ...
            # Built with f32 memsets, then one SBUF->SBUF DMA into the f32r
            # tile (only DMAs may produce f32r-consumed memory).
            selF = cpool.tile([CHUNK, K_MAX * 65], f32)
            nc.vector.memset(selF[:], 0.0)
            diag = selF[:].rearrange("p (r s) -> p r s", s=65)
            nc.vector.memset(diag[:, :, 0:1], 1.0)
            selB = cpool.tile([CHUNK, K_MAX * 65], f32r)
            nc.scalar.dma_start(selB[:], selF[:].bitcast(f32r))

            def sel64_lhsT(r):
                return selB[:, r * K_MAX : (r + 1) * K_MAX]

            def sel32_lhsT(r):
                """(128, 32) slice with its single 1 at column r%32, so the
                4-byte weight load is half as long: out rows land in psum
                half [32*(r//32) : +32] (base partition 0/32 is legal)."""
                s = r * K_MAX + (0 if r < 32 else 32)
                return selB[:, s : s + 32]

            # old-path fold matrix sel[q, j] = (q % 64 == j), from the identity
            sel_f32 = cpool.tile([CHUNK, K_MAX], f32)
            nc.vector.tensor_add(
                sel_f32[:], ident_sb[:, 0:K_MAX], ident_sb[:, K_MAX : 2 * K_MAX]
            )
            sel_sb = sel_f32[:]

            # ---- per-node softmax scales: 1/((softplus(t)+1e-6)*sqrt(D)) ----
            et_sb = cpool.tile([HPC, max(NT, 1)], f32)
            nc.scalar.activation(et_sb[:], temps2_sb, AF.Exp)
            sp_sb = cpool.tile([HPC, max(NT, 1)], f32)
            nc.scalar.activation(sp_sb[:], et_sb[:], AF.Ln, bias=1.0)  # softplus
            u_sb = cpool.tile([HPC, max(NT, 1)], f32)
            # u = (sp + 1e-6) * sqrt(D) = sp*sqrt(D) + 1e-6*sqrt(D)
            nc.scalar.mul(u_sb[:], sp_sb, math.sqrt(HEAD_DIM))
            nc.vector.tensor_scalar_add(u_sb[:], u_sb[:], 1e-6 * math.sqrt(HEAD_DIM))
            rs_sb = cpool.tile([HPC, max(NT, 1)], f32)
            nc.vector.reciprocal(rs_sb[:], u_sb[:])
            ns_sb = cpool.tile([HPC, max(NT, 1)], f32)
            nc.scalar.mul(ns_sb[:], rs_sb[:], -1.0)

            # ---- all tree-node q_depth projections upfront (block-diag) ----
            # all 2*NT little matmuls land in one PSUM tile, then one copy:
            # no PE<->ACT ping-pong at the head of the PE queue.
            qd_all = cpool.tile([2 * HEAD_DIM, max(NT, 1), HPC], f32)
            nc.vector.memset(qd_all[:], 0.0)
            qd_ps = eppool.tile([2 * HEAD_DIM, max(NT, 1), HPC], f32, tag="qd_ps")
            for n in range(NT):
                nc.tensor.matmul(
                    qd_ps[0:HEAD_DIM, n, 0:1],
                    wTI_slice(n), qT_sb[:, 0:1], start=True, stop=True,
                )
                nc.tensor.matmul(
                    qd_ps[HEAD_DIM : 2 * HEAD_DIM, n, 1:2],
                    wTI_slice(n), qT_sb[:, 1:2], start=True, stop=True,
                )
            for n in range(NT):
                nc.scalar.copy(qd_all[0:HEAD_DIM, n, 0:1], qd_ps[0:HEAD_DIM, n, 0:1])
                nc.scalar.copy(
                    qd_all[HEAD_DIM : 2 * HEAD_DIM, n, 1:2],
                    qd_ps[HEAD_DIM : 2 * HEAD_DIM, n, 1:2],
                )

            # ---- cross-node output accumulator (2, 128) PSUM ----
            out_ps = opool.tile([HPC, F], f32)
            n_out_mm = len(tree) + NLC
            out_mm = [0]  # running count, for start/stop flags

            def out_matmul(wT_sb_ap, f_sb_ap):
                nc.tensor.matmul(
                    out_ps[:], wT_sb_ap, f_sb_ap,
                    start=(out_mm[0] == 0), stop=(out_mm[0] == n_out_mm - 1),
                )
                out_mm[0] += 1

            def softmax_weights(s_ps_ap, K, node_i, is_tree):
                """softmax over K free-dim entries of (2, K) logits (pre-scale);
                returns (2, K) SBUF weights; tree weights folded by 1/NT."""
                smax = xpool.tile([HPC, 1], f32, tag="smax")
                nc.vector.reduce_max(smax[:], s_ps_ap, axis=AX.X)
                biast = xpool.tile([HPC, 1], f32, tag="biast")
                ebd = xpool.tile([HPC, K], f32, tag="esb")
                zt = xpool.tile([HPC, 1], f32, tag="zt")
                if is_tree:
                    nc.vector.tensor_scalar_mul(
                        biast[:], smax[:], ns_sb[:, node_i : node_i + 1]
                    )
                    nc.scalar.activation(
                        ebd[:], s_ps_ap, AF.Exp,
                        bias=biast[:], scale=rs_sb[:, node_i : node_i + 1],
                        accum_out=zt[:],
                    )
                else:
                    nc.scalar.mul(biast[:], smax[:], -inv_sqrt_d)
                    nc.scalar.activation(
                        ebd[:], s_ps_ap, AF.Exp, bias=biast[:], scale=inv_sqrt_d,
                        accum_out=zt[:],
                    )
                if is_tree:
                    zs = xpool.tile([HPC, 1], f32, tag="zs")
                    nc.scalar.mul(zs[:], zt[:], float(NT))
                    zt = zs
                rz = xpool.tile([HPC, 1], f32, tag="rz")
                nc.vector.reciprocal(rz[:], zt[:])
                w_sb = xpool.tile([HPC, K], f32, tag="wsb")
                nc.vector.tensor_scalar_mul(w_sb[:], ebd[:], rz[:])
                return w_sb

            def tree_ep_stages(node_i, f_ap, K):
                """Attention epilogue split into 4 stages, one PE touch each.
                Stages run in CONSECUTIVE stream gaps, so each stage's
                scalar/vector inputs were produced a full DMA-unit (~5us)
                earlier and the PE never blocks on a cross-engine wait."""
                st = {}

                def s0():  # fT (F, K) for the logits matmul
                    fT_ps = eppool.tile([F, K_MAX], f32, tag="fT_ps")
                    nc.tensor.transpose(fT_ps[:, 0:K], f_ap, ident_sb[0:K, 0:K])
                    fT_sb = spool.tile([F, K_MAX], f32, tag="fT_sb")
                    nc.scalar.copy(fT_sb[:, 0:K], fT_ps[:, 0:K])
                    st["fT"] = fT_sb

                def s1():  # logits (2, K) + softmax on vec/scalar
                    s_ps = eppool.tile([HPC, K_MAX], f32, tag="s_ps", bufs=2)
                    nc.tensor.matmul(
                        s_ps[:, 0:K], qd_all[:, node_i, :], st["fT"][:, 0:K],
                        start=True, stop=True,
                    )
                    st["w"] = softmax_weights(s_ps[:, 0:K], K, node_i, True)

                def s2():  # wT for the output matmul
                    wT_ps = eppool.tile([K_MAX, HPC], f32, tag="wT_ps")
                    nc.tensor.transpose(
                        wT_ps[0:K, :], st["w"][:], ident_sb[0:HPC, 0:HPC]
                    )
                    wT_sb = spool.tile([K_MAX, HPC], f32, tag="wT_sb")
                    nc.scalar.copy(wT_sb[0:K, :], wT_ps[0:K, :])
                    st["wT"] = wT_sb

                def s3():  # out += wT.T @ f (folded by 1/NT via Z scaling)
                    out_matmul(st["wT"][0:K, :], f_ap)

                return [s0, s1, s2, s3]

            def node_stage_f(ps2, L, CC2):
                """psum (64, 2, F) -> mean-scaled f_sb tile."""
                f_sb = fpool.tile([K_MAX, F], f32, tag="f")
                mean_scale = float(K_MAX) / L
                if CC2 > 1:
                    ha = fpool.tile([K_MAX, F], f32, tag="ha")
                    nc.scalar.mul(ha[:], ps2[:, 0, :], mean_scale)
                    hb = fpool.tile([K_MAX, F], f32, tag="hb")
                    nc.scalar.mul(hb[:], ps2[:, 1, :], mean_scale)
                    nc.vector.tensor_add(f_sb[:], ha[:], hb[:])
                else:
                    nc.scalar.mul(f_sb[:], ps2[:, 0, :], mean_scale)
                return f_sb

            def emit_old_stage_a(node_i, start, L):
                """Fold matmuls + psum->f_sb for an old node; cheap on PE, so
                it runs pre-stream.  The ping-pong epilogue is deferred."""
                nch = L // CHUNK
                vt = oldpath_tiles[start]
                ps2 = apool.tile([K_MAX, 2, F], f32, tag="acc")
                done = 0
                c = 0
                while c < nch:
                    w = 2 if c + 2 <= nch else 1
                    nc.tensor.matmul(
                        ps2[:, 0:w, :], sel_sb, vt[:, c : c + w, :],
                        start=(done == 0), stop=(done + w == nch),
                    )
                    done += w
                    c += w
                return node_stage_f(ps2, L, nch)

            def local_ep_stages():
                st = {}

                def s0():
                    fTl_ps = eppool.tile([F, NLC * CHUNK], f32, tag="fT_ps")
                    for c in range(NLC):
                        nc.tensor.transpose(
                            fTl_ps[:, c * CHUNK : (c + 1) * CHUNK], fl_sb[:, c, :],
                            ident_sb[:],
                        )
                    fTl_sb = spool.tile([F, NLC * CHUNK], f32, tag="fTl_sb")
                    nc.scalar.copy(fTl_sb[:], fTl_ps[:])
                    st["fT"] = fTl_sb

                def s1():
                    sl_ps = eppool.tile([HPC, NLC * CHUNK], f32, tag="s_ps", bufs=2)
                    nc.tensor.matmul(
                        sl_ps[:], qbd_sb, st["fT"][:], start=True, stop=True
                    )
                    st["w"] = softmax_weights(sl_ps[:], n_loc, -1, False)

                def s2():
                    wTl_ps = eppool.tile([CHUNK, NLC, HPC], f32, tag="wT_ps")
                    for c in range(NLC):
                        nc.tensor.transpose(
                            wTl_ps[:, c, :], st["w"][:, c * CHUNK : (c + 1) * CHUNK],
                            ident_sb[0:HPC, 0:HPC],
                        )
                    wTl_sb = spool.tile([CHUNK, NLC, HPC], f32, tag="wTl_sb")
                    nc.scalar.copy(wTl_sb[:], wTl_ps[:])
                    st["wT"] = wTl_sb

                def s3():
                    for c in range(NLC):
                        out_matmul(st["wT"][:, c, :], fl_sb[:, c, :])

                return [s0, s1, s2, s3]

            # ---- old-node fold matmuls pre-stream (the shared acc PSUM slot
            # must be past its last reader before stream node 0 claims it) ----
            old_f = {i: emit_old_stage_a(i, big[i][0], big[i][1]) for i in old}

            # ---- epilogue pipeline: one new epilogue activates per stream
            # gap, and each active epilogue advances one stage per gap ----
            to_activate = [local_ep_stages()]
            to_activate += [
                tree_ep_stages(len(big) + si, small_tiles[si][0 : small[si][1], :],
                               small[si][1])
                for si in range(len(small))
            ]
            to_activate += [tree_ep_stages(i, old_f[i][:], K_MAX) for i in old]
            to_activate.reverse()
            pipe = []

            def pump_stages():
                if to_activate:
                    pipe.append(to_activate.pop())
                for sl in pipe:
                    sl.pop(0)()
                pipe[:] = [sl for sl in pipe if sl]

            # ================= streamed nodes =================
            node_mm_total = {}
            for u in units:
                node_mm_total[u["ni"]] = node_mm_total.get(u["ni"], 0) + u["nr"]
            node_mm_done = {ni: 0 for ni in node_mm_total}
            node_ps = {}

            for k, u in enumerate(units):
                ni = u["ni"]
                ut = unit_tiles[k]
                if u["first"]:
                    acc_ps = apool.tile([K_MAX, 2, F], f32, tag="acc")
                    node_ps[ni] = acc_ps
                ps2 = node_ps[ni]
                done = node_mm_done[ni]
                tot = node_mm_total[ni]
                for j in range(u["nr"]):
                    r = u["r0"] + j
                    nc.tensor.matmul(
                        ps2[:, 0 : u["ncc"], :], sel64_lhsT(r),
                        ut[:, 0 : u["ncc"], j * F : (j + 1) * F],
                        start=(done == 0), stop=(done == tot - 1),
                    )
                    done += 1
                node_mm_done[ni] = done
                # refill the stream pipeline (slot k % STREAM_BUFS now has
                # known readers, so the WAR semaphore is correct)
                if k + STREAM_BUFS < len(units):
                    unit_tiles[k + STREAM_BUFS] = unit_dma(units[k + STREAM_BUFS])
                if u["last"]:
                    start_b, L_b, _d = big[ni]
                    f_sb = node_stage_f(ps2, L_b, 2 if u["ncc"] == 2 else L_b // BLK)
                    to_activate.insert(0, tree_ep_stages(ni, f_sb[:], K_MAX))
                pump_stages()
            # drain remaining epilogue stages (the last node's whole chain)
            while to_activate or pipe:
                pump_stages()

            # ================= final output =================
            acc_sb = spool.tile([HPC, F], f32, tag="acc_sb")
            nc.scalar.copy(acc_sb[:], out_ps[:])
            # head h's output lives at acc_sb[h, h*64:(h+1)*64]; DMA handles the
            # partition-base-1 read that compute engines can't.
            nc.sync.dma_start(o[0:1, :], acc_sb[0:1, 0:HEAD_DIM])
            nc.sync.dma_start(o[1:2, :], acc_sb[1:2, HEAD_DIM : 2 * HEAD_DIM])

    nc.compile()
    return nc


def _make_in_maps(v_tokens, q_new, depth_proj_w, depth_temp, pos):
    nodes = cover_set(pos)
    big = [(st, L, d) for (st, L, d) in nodes if L > K_MAX]
    small = [(st, L, d) for (st, L, d) in nodes if L <= K_MAX]
    tree = big + small
    NT = len(tree)
    OFF, CB_W = _cblob_layout(NT)

    wTI = np.stack(
        [np.eye(HEAD_DIM, dtype=np.float32) + depth_proj_w[d].T for (_, _, d) in tree]
    ) if NT else np.zeros((1, HEAD_DIM, HEAD_DIM), np.float32)
    tsel = np.array([depth_temp[d] for (_, _, d) in tree], np.float32) \
        if NT else np.zeros((1,), np.float32)

    in_maps = []
    for c in range(N_CORES):
        q_c = q_new[0, HPC * c : HPC * (c + 1), :]          # (2, 64)
        cb = np.zeros((CHUNK, CB_W), np.float32)
        cb[:, OFF["ident"] : OFF["ident"] + CHUNK] = np.eye(CHUNK)
        for h in range(HPC):
            cb[h * HEAD_DIM : (h + 1) * HEAD_DIM, OFF["qbd"] + h] = q_c[h]
        cb[0:HEAD_DIM, OFF["qT"] : OFF["qT"] + HPC] = q_c.T
        cb[0:HPC, OFF["temps"] : OFF["temps"] + max(NT, 1)] = tsel[None, :]
        for n in range(max(NT, 1)):
            cb[0:HEAD_DIM, OFF["wTI"] + n * HEAD_DIM : OFF["wTI"] + (n + 1) * HEAD_DIM] = (
                wTI[n] if NT else 0.0
            )
        im = {
            "v": np.ascontiguousarray(
                v_tokens[:, HPC * c : HPC * (c + 1), :]
            ).reshape(NTOK, F),
            "cblob": cb,
        }
        in_maps.append(im)
    return in_maps


def kernel(v_tokens, q_new, depth_proj_w, depth_temp, n_tokens, _profile=False):
    global _last_results
    v_tokens = np.asarray(v_tokens, dtype=np.float32)
    q_new = np.asarray(q_new, dtype=np.float32)
    depth_proj_w = np.asarray(depth_proj_w, dtype=np.float32)
    depth_temp = np.asarray(depth_temp, dtype=np.float32)
    pos = int(n_tokens)

    nc = _build_program(pos)
    in_maps = _make_in_maps(v_tokens, q_new, depth_proj_w, depth_temp, pos)
    res = run_bass_kernel_spmd(
        nc, in_maps, core_ids=list(range(N_CORES)), trace=_profile
    )
    _last_results = res

    out = np.zeros((1, NUM_HEADS, HEAD_DIM), np.float32)
    for c in range(N_CORES):
        out[0, HPC * c : HPC * (c + 1), :] = res.results[c]["o"]
    return out
